# revision 3
# baseline (speedup 1.0000x reference)
"""NeuronMemory retrieval kernel v2 for 8 TRN2 NeuronCores.

Select-then-rescore:
  A. router: 3-way bf16-split matmuls + softmax -> wts [128,16] f32
  B. Q: 3-way bf16-split matmuls (xh@Wh + xl@Wh + xh@Wl), w_n folded into
     Scalar PSUM->SBUF scale-copies, DVE add tree -> Q f32; PE transpose
     (scale 1/sqrt128) -> QT f32r
  C. scores: per (eighth,tile): 8 f32r matmuls [128c x 128p x 512] -> PSUM,
     Scalar copy -> s16 fp16 [128, 4096]
  D. per (e,t): max8 -> top-8 vals; max_index -> positions; + e*4096
  E. per tile: 64 candidates -> +tiny*j pack (unique f32) -> top-12 prerank
     (max8 + match_replace + max8, is_equal-sum -> 12 global idx)
  F. KV gather: 12 indirect DMA calls -> slots (row = K f32 512B | V fp16 2KB)
  G. rescore: r_j = (Q . K_j)/sqrt(128) exact f32 on DVE
  H. top-8-of-12 mask softmax on r, weighted fp16 V-slot sum -> out f32
Sharding: data-parallel over tokens; tables replicated; no collectives.
"""
import numpy as np
import ml_dtypes

import concourse.bacc as bacc
import concourse.bass as bass
import concourse.mybir as mybir
from concourse.tile import TileContext
from concourse.bass_utils import run_bass_kernel_spmd

P = 128
D_MODEL = 1024
RANK = 128
N_COMPRESS = 16
N_KNOWLEDGE = 32768
B, S = 2, 2048
N_CORES = 8
TOK_PER_CORE = (B * S) // N_CORES      # 512
N_TILES = TOK_PER_CORE // P            # 4
N_DC = D_MODEL // P                    # 8
N_E = 8                                # eighths
EW = N_KNOWLEDGE // N_E                # 4096
N_CH = EW // 512                       # 8 chunks per eighth
NCAND = N_E * 8                        # 64 pair-candidates per token
NRES = 10                              # rescored pairs (20 elements)
KPW = 1024                             # u16 cols per K-quad row: 4 x 256 (K f32)
QW4 = 1024                             # quads per eighth
PW = 2048                              # pairs per eighth
SCALE = 1.0 / np.sqrt(np.float32(RANK))

f32 = mybir.dt.float32
f32r = mybir.dt.float32r
fp16 = mybir.dt.float16
bf16 = mybir.dt.bfloat16
u16 = mybir.dt.uint16
u32 = mybir.dt.uint32
AF = mybir.ActivationFunctionType
OP = mybir.AluOpType
AX = mybir.AxisListType


def _build(dbg=False):
    nc = bacc.Bacc("TRN2", target_bir_lowering=False, debug=False, num_devices=N_CORES)

    xh = nc.declare_dram_parameter("xh", [P, N_DC * TOK_PER_CORE], bf16, isOutput=False)
    xl = nc.declare_dram_parameter("xl", [P, N_DC * TOK_PER_CORE], bf16, isOutput=False)
    rwh = nc.declare_dram_parameter("rwh", [P, N_DC * N_COMPRESS], bf16, isOutput=False)
    rwl = nc.declare_dram_parameter("rwl", [P, N_DC * N_COMPRESS], bf16, isOutput=False)
    Wh = nc.declare_dram_parameter("Wh", [4 * N_DC * P, 512], bf16, isOutput=False)
    Wl = nc.declare_dram_parameter("Wl", [4 * N_DC * P, 512], bf16, isOutput=False)
    KT = nc.declare_dram_parameter("KT", [P, N_KNOWLEDGE], f32r, isOutput=False)
    KP = nc.declare_dram_parameter("KP", [N_KNOWLEDGE // 4, KPW], u16, isOutput=False)
    V16 = nc.declare_dram_parameter("V16", [N_KNOWLEDGE, 1024], u16, isOutput=False)
    ident = nc.declare_dram_parameter("ident", [P, P], f32, isOutput=False)
    packc = nc.declare_dram_parameter("packc", [P, NCAND], f32, isOutput=False)
    offc = nc.declare_dram_parameter("offc", [P, NCAND], f32, isOutput=False)
    out = nc.declare_dram_parameter("out", [TOK_PER_CORE, D_MODEL], f32, isOutput=True)
    if dbg:
        d_wts = nc.declare_dram_parameter("d_wts", [P, N_TILES * N_COMPRESS], f32, isOutput=True)
        d_q = nc.declare_dram_parameter("d_q", [P, N_TILES * RANK], f32, isOutput=True)
        d_cv = nc.declare_dram_parameter("d_cv", [P, N_TILES * NCAND], f32, isOutput=True)
        d_ci = nc.declare_dram_parameter("d_ci", [P, N_TILES * NCAND], f32, isOutput=True)
        d_pi = nc.declare_dram_parameter("d_pi", [P, N_TILES * NRES], f32, isOutput=True)
        d_r = nc.declare_dram_parameter("d_r", [P, N_TILES * 4 * NRES], f32, isOutput=True)
        d_w = nc.declare_dram_parameter("d_w", [P, N_TILES * 4 * NRES], f32, isOutput=True)

    Wh_v = Wh.rearrange("(g dc p) n -> g dc p n", g=4, dc=N_DC)
    Wl_v = Wl.rearrange("(g dc p) n -> g dc p n", g=4, dc=N_DC)

    with TileContext(nc) as tc:
        with (
            tc.tile_pool(name="const", bufs=1) as cpool,
            tc.tile_pool(name="kt", bufs=2) as ktpool,
            tc.tile_pool(name="sc", bufs=2) as scpool,
            tc.tile_pool(name="wld", bufs=1) as wpool,
            tc.tile_pool(name="kp", bufs=2) as kppool,
            tc.tile_pool(name="vg", bufs=1) as vpool,
            tc.tile_pool(name="acc", bufs=1) as apool,
            tc.tile_pool(name="small", bufs=2) as spool,
            tc.tile_pool(name="ps_big", bufs=3, space="PSUM") as psb,
            tc.tile_pool(name="ps_small", bufs=1, space="PSUM") as pss,
        ):
            # ---------- persistent loads ----------
            xh_sb = cpool.tile([P, N_DC * TOK_PER_CORE], bf16)
            xl_sb = cpool.tile([P, N_DC * TOK_PER_CORE], bf16)
            rwh_sb = cpool.tile([P, N_DC * N_COMPRESS], bf16)
            rwl_sb = cpool.tile([P, N_DC * N_COMPRESS], bf16)
            id_sb = cpool.tile([P, P], f32)
            packc_sb = cpool.tile([P, NCAND], f32)
            offc_sb = cpool.tile([P, NCAND], f32)
            nc.sync.dma_start(out=xh_sb[:], in_=xh[:])
            nc.sync.dma_start(out=xl_sb[:], in_=xl[:])
            nc.sync.dma_start(out=rwh_sb[:], in_=rwh[:])
            nc.sync.dma_start(out=rwl_sb[:], in_=rwl[:])
            nc.sync.dma_start(out=id_sb[:], in_=ident[:])
            nc.sync.dma_start(out=packc_sb[:], in_=packc[:])
            nc.sync.dma_start(out=offc_sb[:], in_=offc[:])

            wts_sb = cpool.tile([P, N_TILES * N_COMPRESS], f32)
            Q_sb = cpool.tile([P, N_TILES * RANK], f32)     # [tok, r]
            QT_sb = cpool.tile([P, N_TILES * P], f32r)      # [r, tok] (scaled 1/sqrt128)
            cv_sb = cpool.tile([P, N_TILES * NCAND], fp16)
            ci_sb = cpool.tile([P, N_TILES * NCAND], u32)

            def tok(t):
                return slice(t * P, (t + 1) * P)

            # ---------- A: router softmax (bf16 3-split) ----------
            for t in range(N_TILES):
                rps = pss.tile([P, N_COMPRESS], f32, space="PSUM", tag="pss")
                i, nmm = 0, 3 * N_DC
                for dc in range(N_DC):
                    xs = slice(dc * TOK_PER_CORE + t * P, dc * TOK_PER_CORE + (t + 1) * P)
                    rs = slice(dc * N_COMPRESS, (dc + 1) * N_COMPRESS)
                    for lhs, rhs in ((xh_sb, rwh_sb), (xl_sb, rwh_sb), (xh_sb, rwl_sb)):
                        nc.tensor.matmul(out=rps[:], lhsT=lhs[:, xs], rhs=rhs[:, rs],
                                         start=(i == 0), stop=(i == nmm - 1))
                        i += 1
                w = wts_sb[:, t * N_COMPRESS:(t + 1) * N_COMPRESS]
                mx = spool.tile([P, 1], f32, tag="mx")
                sm = spool.tile([P, 1], f32, tag="sm")
                ex = spool.tile([P, N_COMPRESS], f32, tag="ex")
                nc.vector.tensor_reduce(out=mx[:], in_=rps[:], op=OP.max, axis=AX.X)
                nc.vector.tensor_scalar(out=ex[:], in0=rps[:], scalar1=mx[:, :1],
                                        scalar2=None, op0=OP.subtract)
                nc.scalar.activation(out=ex[:], in_=ex[:], func=AF.Exp, accum_out=sm[:, :1])
                rcp = spool.tile([P, 1], f32, tag="rcp")
                nc.vector.reciprocal(out=rcp[:], in_=sm[:, :1])
                nc.vector.tensor_scalar(out=w, in0=ex[:], scalar1=rcp[:, :1],
                                        scalar2=None, op0=OP.mult)

            # ---------- B: Q projection (bf16 3-split), fold w_n at PSUM copy ----------
            for g in range(4):
                wh_t, wl_t = {}, {}
                for dc in range(N_DC):
                    wh_t[dc] = wpool.tile([P, 512], bf16, tag=f"wh{dc}", name=f"wh_{g}_{dc}")
                    wl_t[dc] = wpool.tile([P, 512], bf16, tag=f"wl{dc}", name=f"wl_{g}_{dc}")
                    nc.sync.dma_start(out=wh_t[dc][:], in_=Wh_v[g, dc])
                    nc.sync.dma_start(out=wl_t[dc][:], in_=Wl_v[g, dc])
                for t in range(N_TILES):
                    yps = psb.tile([P, 512], f32, space="PSUM", tag="ps", name=f"y_{g}_{t}")
                    i, nmm = 0, 3 * N_DC
                    for dc in range(N_DC):
                        xs = slice(dc * TOK_PER_CORE + t * P, dc * TOK_PER_CORE + (t + 1) * P)
                        for lhs, rhs in ((xh_sb, wh_t[dc]), (xl_sb, wh_t[dc]), (xh_sb, wl_t[dc])):
                            nc.tensor.matmul(out=yps[:], lhsT=lhs[:, xs], rhs=rhs[:],
                                             start=(i == 0), stop=(i == nmm - 1))
                            i += 1
                    for n in range(4):
                        ncomp = g * 4 + n
                        ysc = spool.tile([P, RANK], f32, tag=f"ysc{n % 2}",
                                         name=f"ysc_{g}_{t}_{n}")
                        nc.scalar.activation(
                            out=ysc[:], in_=yps[:, n * RANK:(n + 1) * RANK], func=AF.Copy,
                            scale=wts_sb[:, t * N_COMPRESS + ncomp:t * N_COMPRESS + ncomp + 1])
                        q = Q_sb[:, t * RANK:(t + 1) * RANK]
                        if ncomp == 0:
                            nc.vector.tensor_copy(out=q, in_=ysc[:])
                        else:
                            nc.vector.tensor_tensor(out=q, in0=q, in1=ysc[:], op=OP.add)

            for t in range(N_TILES):
                tps = pss.tile([P, P], f32, space="PSUM", tag="pss", name=f"tps_{t}")
                nc.tensor.transpose(out=tps[:], in_=Q_sb[:, t * RANK:(t + 1) * RANK],
                                    identity=id_sb[:])
                nc.scalar.activation(out=QT_sb[:, tok(t)], in_=tps[:], func=AF.Copy,
                                     scale=float(SCALE))

            if dbg:
                nc.sync.dma_start(out=d_wts[:], in_=wts_sb[:])
                nc.sync.dma_start(out=d_q[:], in_=Q_sb[:])

            # ---------- E/F/G/H for one tile ----------
            def tile_tail(t):
                cv = cv_sb[:, t * NCAND:(t + 1) * NCAND]
                ci = ci_sb[:, t * NCAND:(t + 1) * NCAND]
                cif = spool.tile([P, NCAND], f32, tag="cif", name=f"cif_{t}")
                nc.vector.tensor_copy(out=cif[:], in_=ci)
                nc.vector.tensor_tensor(out=cif[:], in0=cif[:], in1=offc_sb[:], op=OP.add)
                cf = spool.tile([P, NCAND], f32, tag="cf", name=f"cf_{t}")
                nc.vector.tensor_copy(out=cf[:], in_=cv)
                nc.vector.tensor_tensor(out=cf[:], in0=cf[:], in1=packc_sb[:], op=OP.add)
                if dbg:
                    nc.sync.dma_start(out=d_cv[:, t * NCAND:(t + 1) * NCAND], in_=cf[:])
                    nc.sync.dma_start(out=d_ci[:, t * NCAND:(t + 1) * NCAND], in_=cif[:])
                v8a = spool.tile([P, 8], f32, tag="v8a", name=f"v8a_{t}")
                v8b = spool.tile([P, 8], f32, tag="v8b", name=f"v8b_{t}")
                mrep = spool.tile([P, NCAND], f32, tag="mrep", name=f"mrep_{t}")
                nc.vector.max(out=v8a[:], in_=cf[:])
                nc.vector.match_replace(out=mrep[:], in_to_replace=v8a[:], in_values=cf[:],
                                        imm_value=-1e30)
                nc.vector.max(out=v8b[:], in_=mrep[:])
                pv = spool.tile([P, NRES], f32, tag="pv", name=f"pv_{t}")
                nc.vector.tensor_copy(out=pv[:, 0:8], in_=v8a[:])
                nc.vector.tensor_copy(out=pv[:, 8:NRES], in_=v8b[:, 0:NRES - 8])
                pidx = spool.tile([P, NRES], f32, tag="pidx", name=f"pidx_{t}")
                junk = spool.tile([P, NCAND], f32, tag="junk", name=f"junk_{t}")
                for j in range(NRES):
                    nc.vector.scalar_tensor_tensor(
                        out=junk[:], in0=cf[:], scalar=pv[:, j:j + 1], in1=cif[:],
                        op0=OP.is_equal, op1=OP.mult,
                        accum_out=pidx[:, j:j + 1])
                pidx_u = spool.tile([P, NRES], u32, tag="pidxu", name=f"pidxu_{t}")
                nc.vector.tensor_copy(out=pidx_u[:], in_=pidx[:])
                if dbg:
                    nc.sync.dma_start(out=d_pi[:, t * NRES:(t + 1) * NRES], in_=pidx[:])

                # gather NRES K-pair rows (1KB each)
                kp_sb = kppool.tile([P, NRES * KPW], u16, tag="kp", name=f"kp_{t}")
                for j in range(NRES):
                    nc.gpsimd.indirect_dma_start(
                        out=kp_sb[:, j * KPW:(j + 1) * KPW],
                        out_offset=None,
                        in_=KP[:],
                        in_offset=bass.IndirectOffsetOnAxis(ap=pidx_u[:, j:j + 1], axis=0),
                    )

                # rescore 2*NRES elements
                NEL = 4 * NRES
                r = spool.tile([P, NEL], f32, tag="r", name=f"r_{t}")
                for h in range(4):
                    for j in range(NRES):
                        ko = j * KPW + h * 256
                        kj = kp_sb[:, ko:ko + 256].bitcast(f32)
                        m = h * NRES + j
                        prod = spool.tile([P, RANK], f32, tag=f"prod{m % 2}",
                                          name=f"prod_{t}_{m}")
                        nc.vector.tensor_tensor(out=prod[:],
                                                in0=Q_sb[:, t * RANK:(t + 1) * RANK],
                                                in1=kj, op=OP.mult)
                        nc.vector.tensor_reduce(out=r[:, m:m + 1], in_=prod[:],
                                                op=OP.add, axis=AX.X)
                nc.vector.tensor_scalar(out=r[:], in0=r[:], scalar1=float(SCALE),
                                        scalar2=None, op0=OP.mult)
                if dbg:
                    nc.sync.dma_start(out=d_r[:, t * NEL:(t + 1) * NEL], in_=r[:])

                # V16 rows are quad-interleaved: row(m) = 4*pid + h (h-major order)
                gk = spool.tile([P, NEL], f32, tag="gk", name=f"gk_{t}")
                nc.vector.tensor_scalar(out=gk[:, 0:NRES], in0=pidx[:], scalar1=4.0,
                                        scalar2=None, op0=OP.mult)
                for h in range(1, 4):
                    nc.vector.tensor_scalar(out=gk[:, h * NRES:(h + 1) * NRES],
                                            in0=gk[:, 0:NRES], scalar1=float(h),
                                            scalar2=None, op0=OP.add)

                # top-8-of-NEL mask softmax on rescored values
                r8 = spool.tile([P, 8], f32, tag="r8", name=f"r8_{t}")
                nc.vector.max(out=r8[:], in_=r[:])
                negm = spool.tile([P, 1], f32, tag="negm", name=f"negm_{t}")
                nc.vector.tensor_scalar(out=negm[:], in0=r8[:, 0:1], scalar1=-1.0,
                                        scalar2=None, op0=OP.mult)
                e_t = spool.tile([P, NEL], f32, tag="et", name=f"et_{t}")
                nc.scalar.activation(out=e_t[:], in_=r[:], func=AF.Exp,
                                     bias=negm[:, :1], scale=1.0)
                z = spool.tile([P, 1], f32, tag="z", name=f"z_{t}")
                msk = spool.tile([P, NEL], f32, tag="msk", name=f"msk_{t}")
                nc.vector.tensor_scalar(out=msk[:], in0=r[:], scalar1=r8[:, 7:8],
                                        scalar2=None, op0=OP.is_ge)
                nc.vector.tensor_tensor(out=e_t[:], in0=e_t[:], in1=msk[:], op=OP.mult)
                nc.vector.tensor_reduce(out=z[:], in_=e_t[:], op=OP.add, axis=AX.X)
                rz = spool.tile([P, 1], f32, tag="rz", name=f"rz_{t}")
                nc.vector.reciprocal(out=rz[:], in_=z[:, :1])
                wn = spool.tile([P, NEL], f32, tag="wn", name=f"wn_{t}")
                nc.vector.tensor_scalar(out=wn[:], in0=e_t[:], scalar1=rz[:, :1],
                                        scalar2=None, op0=OP.mult)
                if dbg:
                    nc.sync.dma_start(out=d_w[:, t * NEL:(t + 1) * NEL], in_=wn[:])

                # final-8: weights w8f and key idx g8 via is_equal-sum on unique fp32 r
                w8f = spool.tile([P, 8], f32, tag="w8f", name=f"w8f_{t}")
                g8 = spool.tile([P, 8], f32, tag="g8", name=f"g8_{t}")
                junk2 = spool.tile([P, NEL], f32, tag="junk2", name=f"junk2_{t}")
                for j in range(8):
                    nc.vector.scalar_tensor_tensor(
                        out=junk2[:], in0=r[:], scalar=r8[:, j:j + 1], in1=gk[:],
                        op0=OP.is_equal, op1=OP.mult,
                        accum_out=g8[:, j:j + 1])
                    nc.vector.scalar_tensor_tensor(
                        out=junk2[:], in0=r[:], scalar=r8[:, j:j + 1], in1=wn[:],
                        op0=OP.is_equal, op1=OP.mult,
                        accum_out=w8f[:, j:j + 1])
                g8u = spool.tile([P, 8], u32, tag="g8u", name=f"g8u_{t}")
                nc.vector.tensor_copy(out=g8u[:], in_=g8[:])

                # gather 8 V rows, weighted sum -> out
                v_sb = vpool.tile([P, 8 * 1024], u16, tag="vg", name=f"vg_{t}")
                for j in range(8):
                    nc.gpsimd.indirect_dma_start(
                        out=v_sb[:, j * 1024:(j + 1) * 1024],
                        out_offset=None,
                        in_=V16[:],
                        in_offset=bass.IndirectOffsetOnAxis(ap=g8u[:, j:j + 1], axis=0),
                    )
                vsc = {}
                for j in range(8):
                    vj = v_sb[:, j * 1024:(j + 1) * 1024].bitcast(fp16)
                    vsc[j] = apool.tile([P, D_MODEL], fp16, tag=f"vsc{j}", name=f"vsc_{t}_{j}")
                    nc.vector.tensor_scalar(out=vsc[j][:], in0=vj, scalar1=w8f[:, j:j + 1],
                                            scalar2=None, op0=OP.mult)
                for k in range(4):
                    nc.vector.tensor_tensor(out=vsc[2 * k][:], in0=vsc[2 * k][:],
                                            in1=vsc[2 * k + 1][:], op=OP.add)
                nc.vector.tensor_tensor(out=vsc[0][:], in0=vsc[0][:], in1=vsc[2][:], op=OP.add)
                acc = apool.tile([P, D_MODEL], f32, tag="accf", name=f"acc_{t}")
                nc.vector.tensor_tensor(out=acc[:], in0=vsc[4][:], in1=vsc[6][:], op=OP.add)
                nc.vector.tensor_tensor(out=acc[:], in0=acc[:], in1=vsc[0][:], op=OP.add)
                nc.sync.dma_start(out=out[t * P:(t + 1) * P, :], in_=acc[:])

            # ---------- C/D: scores + per-eighth exact top-8 ----------
            for e in range(N_E):
                kte = ktpool.tile([P, EW], f32r, tag="kte")
                nc.sync.dma_start(out=kte[:], in_=KT[:, e * EW:(e + 1) * EW])
                for t in range(N_TILES):
                    s16 = scpool.tile([P, EW], fp16, tag="s16")
                    for c in range(N_CH // 2):
                        sps = psb.tile([P, 1024], f32, space="PSUM", tag="ps")
                        for h in range(2):
                            nc.tensor.matmul(
                                out=sps[:, h * 512:(h + 1) * 512],
                                lhsT=QT_sb[:, tok(t)],
                                rhs=kte[:, c * 1024 + h * 512:c * 1024 + (h + 1) * 512],
                                start=True, stop=True,
                            )
                        nc.scalar.copy(out=s16[:, c * 1024:(c + 1) * 1024], in_=sps[:])
                    # quad tree: L1[i] = max(s16[i], s16[i+2048]); L2[i] = max(L1[i], L1[i+1024])
                    L1 = scpool.tile([P, PW], fp16, tag="L1")
                    nc.vector.tensor_tensor(out=L1[:], in0=s16[:, 0:PW], in1=s16[:, PW:EW],
                                            op=OP.max)
                    L2 = scpool.tile([P, QW4], fp16, tag="L2")
                    nc.vector.tensor_tensor(out=L2[:], in0=L1[:, 0:QW4], in1=L1[:, QW4:PW],
                                            op=OP.max)
                    v8 = cv_sb[:, t * NCAND + e * 8:t * NCAND + e * 8 + 8]
                    i8 = ci_sb[:, t * NCAND + e * 8:t * NCAND + e * 8 + 8]
                    nc.vector.max(out=v8, in_=L2[:])
                    nc.vector.max_index(out=i8, in_max=v8, in_values=L2[:])
                    if e == N_E - 1:
                        tile_tail(t)

    nc.compile()
    return nc


_NC_CACHE = {}


def _get_nc(dbg=False):
    if dbg not in _NC_CACHE:
        _NC_CACHE[dbg] = _build(dbg)
    return _NC_CACHE[dbg]


def _bf16(a):
    return a.astype(ml_dtypes.bfloat16)


def _prep_in_maps(x, router_w, compress_neurons, knowledge_K, knowledge_V):
    x = np.asarray(x, dtype=np.float32).reshape(B * S, D_MODEL)
    rwT = np.ascontiguousarray(np.asarray(router_w, dtype=np.float32).T)
    rw_r = np.ascontiguousarray(
        rwT.reshape(N_DC, P, N_COMPRESS).transpose(1, 0, 2).reshape(P, N_DC * N_COMPRESS))
    rwh = _bf16(rw_r)
    rwl = _bf16(rw_r - rwh.astype(np.float32))
    cn = np.asarray(compress_neurons, dtype=np.float32)
    Wg = np.ascontiguousarray(
        cn.reshape(4, 4, N_DC, P, RANK).transpose(0, 2, 3, 1, 4).reshape(4 * N_DC * P, 4 * RANK))
    Wh = _bf16(Wg)
    Wl = _bf16(Wg - Wh.astype(np.float32))
    K = np.asarray(knowledge_K, dtype=np.float32)
    KT = np.ascontiguousarray(K.T)
    V = np.asarray(knowledge_V, dtype=np.float32)
    # K-quad table: quad (e*QW4 + i) = keys e*EW + i + QW4*h, h in 0..3
    KPE = np.ascontiguousarray(K).view(np.uint16).reshape(N_E, 4, QW4, 256)
    KP = np.ascontiguousarray(KPE.transpose(0, 2, 1, 3).reshape(N_KNOWLEDGE // 4, KPW))
    V16 = np.ascontiguousarray(
        V.astype(np.float16).view(np.uint16).reshape(N_E, 4, QW4, 1024)
        .transpose(0, 2, 1, 3).reshape(N_KNOWLEDGE, 1024))
    ident = np.eye(P, dtype=np.float32)
    packc = np.broadcast_to(
        (np.arange(NCAND, dtype=np.float32) * np.float32(2 ** -25))[None, :], (P, NCAND)).copy()
    offc = np.broadcast_to(
        ((np.arange(NCAND, dtype=np.float32) // 8).astype(np.float32) * np.float32(QW4))[None, :],
        (P, NCAND)).copy()

    in_maps = []
    for c in range(N_CORES):
        xs = x[c * TOK_PER_CORE:(c + 1) * TOK_PER_CORE]
        xT = np.ascontiguousarray(
            xs.T.reshape(N_DC, P, TOK_PER_CORE).transpose(1, 0, 2).reshape(P, N_DC * TOK_PER_CORE))
        xhn = _bf16(xT)
        xln = _bf16(xT - xhn.astype(np.float32))
        in_maps.append(dict(xh=xhn, xl=xln, rwh=rwh, rwl=rwl, Wh=Wh, Wl=Wl,
                            KT=KT, KP=KP, V16=V16, ident=ident, packc=packc, offc=offc))
    return in_maps


def _ensure_ntff_hook():
    import sys as _sys
    import types as _types
    if "antenv.axon_hooks" in _sys.modules:
        return
    try:
        import antenv.axon_hooks  # noqa: F401
        return
    except ImportError:
        pass
    mod = _types.ModuleType("antenv.axon_hooks")
    _state = {"hook": None}
    mod.set_axon_ntff_profile_hook = lambda h: _state.__setitem__("hook", h)
    mod.get_axon_ntff_profile_hook = lambda: _state["hook"]
    _sys.modules["antenv.axon_hooks"] = mod
    try:
        from trn_agent_boot.trn_boot import _ntff_profile_via_ctypes
        mod.set_axon_ntff_profile_hook(_ntff_profile_via_ctypes("/opt/axon/libaxon_pjrt.so"))
    except Exception:
        pass


def _run(inputs, trace=False, dbg=False):
    if trace:
        _ensure_ntff_hook()
    nc = _get_nc(dbg)
    in_maps = _prep_in_maps(**inputs)
    res = run_bass_kernel_spmd(nc, in_maps, core_ids=list(range(N_CORES)), trace=trace)
    out = np.concatenate([res.results[c]["out"] for c in range(N_CORES)], axis=0)
    return out.reshape(B, S, D_MODEL), res


def kernel(x, router_w, compress_neurons, knowledge_K, knowledge_V):
    out, _ = _run(dict(x=x, router_w=router_w, compress_neurons=compress_neurons,
                       knowledge_K=knowledge_K, knowledge_V=knowledge_V))
    return out
